# revision 36
# baseline (speedup 1.0000x reference)
"""LoRA linear layer (out = x @ (W + B@A).T + bias) on 8 trn2 NeuronCores.

Strategy: data-parallel over tokens (B*S = 8192 -> 1024 tokens/core).
All layout work (transposes, dtype casts) happens on the host so the
device does nothing but matmuls:

  - x shard arrives pre-transposed: a fp8(e4m3) copy of the first K8
    contraction dims in DoubleRow pair layout, and a fp16 copy of the
    remaining dims. Both DMA straight into resident SBUF.
  - W arrives pre-transposed (k-major) and pre-cast: fp8 (DoubleRow
    [SW-interleave] pair layout, scaled by 64) for k < K8, fp16
    (scaled by 64) for k >= K8. Streamed per 128-row output block.
  - Per output block m and token chunk n: psum[o=128, t=512]
    accumulates G8 DoubleRow fp8 matmuls (256 k-dims each, 2x rate),
    KT16 fp16 matmuls (128 k-dims each), and one rank-16 LoRA matmul
    from B.T and U = (64*A) @ x.T.
  - Scalar engine evicts psum with the bias added and the 1/64 weight
    scale removed; plain contiguous DMA writes out.T [o, t] rows and
    the host transposes each shard back.

fp8 on only half the contraction dims keeps the max relative error
~1.8e-2 (measured on the reference data) while cutting PE time ~25%.
"""

import sys

sys.path.insert(0, "/opt/trn_rl_repo")

import numpy as np
import ml_dtypes

import concourse.bass as bass  # noqa: F401
import concourse.bacc as bacc
import concourse.tile as tile
from concourse import mybir, bass_utils
from contextlib import ExitStack

P = 128
N_CORES = 8

# Full problem shapes (hardcoded per contract).
B_FULL, S_FULL, D_IN, D_OUT, R = 4, 2048, 4096, 4096, 16
T_CORE = (B_FULL * S_FULL) // N_CORES  # 1024 tokens per core
MT = D_OUT // P  # 32 output row blocks
NCH = 512  # token chunk (one psum bank)
NT = T_CORE // NCH  # 2 chunks
K8 = 2304  # leading contraction dims done in fp8 DoubleRow
SCALE = 64.0  # fp8 weight scale (keeps 64*W in e4m3 normal range)
SWIL = True  # DoubleRowSwInterleave (contiguous ldweights) vs DoubleRow

FP8NP = ml_dtypes.float8_e4m3fn


def build_nc(T=T_CORE, k8=K8, swil=SWIL, fp16=None):
    """Per-core bass program; all cores run it on different token shards.

    fp16 kwarg is accepted for test.py compat: fp16=True/None keeps the
    default hybrid config, fp16-only can be forced with k8=0.
    """
    F32 = mybir.dt.float32
    F16 = mybir.dt.float16
    F8 = mybir.dt.float8e4
    G8 = k8 // 256
    KT16 = (D_IN - k8) // P
    DRMODE = (
        mybir.MatmulPerfMode.DoubleRowSwInterleave
        if swil
        else mybir.MatmulPerfMode.DoubleRow
    )
    IDENT = mybir.ActivationFunctionType.Identity
    HT = T // 2

    nc = bacc.Bacc("TRN2", target_bir_lowering=False, debug=False)
    if G8:
        # pair-interleaved moving layout: the two fp8 elements of each
        # DoubleRow pair sit adjacent in SBUF so the PE can stream both
        # per cycle
        x8_d = nc.dram_tensor("x8", [P, G8, T, 2], F8, kind="ExternalInput").ap()
        if swil:
            w8_d = nc.dram_tensor("w8", [MT, P, G8, 2 * P], F8, kind="ExternalInput").ap()
        else:
            w8_d = nc.dram_tensor("w8", [MT, P, G8, 2, P], F8, kind="ExternalInput").ap()
        at8_d = nc.dram_tensor("at8", [P, G8, 2, R], F8, kind="ExternalInput").ap()
    if KT16:
        x16_d = nc.dram_tensor("x16", [P, KT16, T], F16, kind="ExternalInput").ap()
        w16_d = nc.dram_tensor("w16", [MT, P, KT16, P], F16, kind="ExternalInput").ap()
        at16_d = nc.dram_tensor("at16", [P, KT16, R], F16, kind="ExternalInput").ap()
    bt_d = nc.dram_tensor("bt", [R, D_OUT], F16, kind="ExternalInput").ap()
    bias_d = nc.dram_tensor("bias_r", [P, MT], F32, kind="ExternalInput").ap()
    out_d = nc.dram_tensor("out", [D_OUT, T], F16, kind="ExternalOutput").ap()

    with tile.TileContext(nc) as tc, ExitStack() as ctx:
        const = ctx.enter_context(tc.tile_pool(name="const", bufs=1))
        if G8:
            # one tile per 256-dim k-group so matmuls only wait on their own
            # DMA slice, not the whole x transfer
            xt8 = [const.tile([P, T, 2], F8, name=f"x8g{g}") for g in range(G8)]
            at8s = const.tile([P, G8, 2, R], F8)
        if KT16:
            xt16 = [const.tile([P, T], F16, name=f"x16j{j}") for j in range(KT16)]
            at16s = const.tile([P, KT16, R], F16)
        bt_sb = const.tile([R, D_OUT], F16)
        bias_sb = const.tile([P, MT], F32)
        u_sb = const.tile([R, T], F16)

        up_psum = ctx.enter_context(tc.tile_pool(name="upps", bufs=2, space="PSUM"))
        mm_psum = ctx.enter_context(tc.tile_pool(name="mmps", bufs=6, space="PSUM"))
        w8_pool = ctx.enter_context(tc.tile_pool(name="w8p", bufs=8))
        w16_pool = ctx.enter_context(tc.tile_pool(name="w16p", bufs=8))
        ob_pool = ctx.enter_context(tc.tile_pool(name="obp", bufs=6))

        w8_tiles, w16_tiles = {}, {}

        def dma_w8(m):
            t8 = w8_pool.tile(
                [P, G8, 2 * P] if swil else [P, G8, 2, P], F8, tag="w8", name="w8t"
            )
            nc.scalar.dma_start(t8[:], w8_d[m])
            w8_tiles[m] = t8

        def dma_w16(m):
            t16 = w16_pool.tile([P, KT16, P], F16, tag="w16", name="w16t")
            nc.sync.dma_start(t16[:], w16_d[m])
            w16_tiles[m] = t16

        # ---- prologue DMAs: x split by k-range across the three queues so
        # every transfer keeps 2KB+ contiguous per-partition lines; the
        # first output block's matmuls chase the arriving k-tiles ----
        nc.sync.dma_start(bias_sb[:], bias_d[:])
        nc.sync.dma_start(bt_sb[:], bt_d[:])
        if KT16:
            nc.sync.dma_start(at16s[:], at16_d[:])
        WM = min(2, MT)
        if G8:
            nc.sync.dma_start(at8s[:], at8_d[:])
            for m_ in range(WM):
                dma_w8(m_)
        if KT16:
            for m_ in range(WM):
                dma_w16(m_)
        if G8:
            for g in range(G8 - 1):
                nc.scalar.dma_start(xt8[g][:], x8_d[:, g])
            nc.gpsimd.dma_start(xt8[G8 - 1][:], x8_d[:, G8 - 1])
        if KT16:
            JH = KT16 // 2
            for j in range(JH):
                nc.sync.dma_start(xt16[j][:], x16_d[:, j])
            for j in range(JH, KT16):
                nc.gpsimd.dma_start(xt16[j][:], x16_d[:, j])

        # ---- U = (64*A) @ x^T, [R, T], one token chunk ----
        def emit_u(n):
            nsl = slice(n * NCH, (n + 1) * NCH)
            ups = up_psum.tile([R, NCH], F32, tag="up", name="ups")
            first = True
            for g in range(G8):
                for s_ in range(2):
                    nc.tensor.matmul(
                        ups[:],
                        at8s[:, g, s_, :],
                        xt8[g][:, nsl, s_],
                        start=first,
                        stop=(not KT16) and g == G8 - 1 and s_ == 1,
                    )
                    first = False
            for j in range(KT16):
                nc.tensor.matmul(
                    ups[:],
                    at16s[:, j, :],
                    xt16[j][:, nsl],
                    start=first,
                    stop=j == KT16 - 1,
                )
                first = False
            nc.vector.tensor_copy(u_sb[:, nsl], ups[:])

        # ---- one (m, n) output tile: 24 accumulating matmuls + eviction ----
        def emit_main(m, n):
            issued = w8_tiles if G8 else w16_tiles
            if n == 0:
                for mw in (m + 2, m + 3):
                    if mw < MT and mw not in issued:
                        if G8:
                            dma_w8(mw)
                        if KT16:
                            dma_w16(mw)
                        break
            nsl = slice(n * NCH, (n + 1) * NCH)
            ps = mm_psum.tile([P, NCH], F32, tag="mm", name="mps")
            for g in range(G8):
                w8t = w8_tiles[m]
                w_ap = w8t[:, g, :] if swil else w8t[:, g, :, :]
                nc.tensor.matmul(
                    ps[:],
                    w_ap,
                    xt8[g][:, nsl, :].rearrange("p t s -> p s t"),
                    start=g == 0,
                    stop=False,
                    perf_mode=DRMODE,
                )
            for j in range(KT16):
                nc.tensor.matmul(
                    ps[:],
                    w16_tiles[m][:, j, :],
                    xt16[j][:, nsl],
                    start=(not G8) and j == 0,
                    stop=False,
                )
            return ps

        def emit_fin(m, n, ps):
            msl = slice(m * P, (m + 1) * P)
            nsl = slice(n * NCH, (n + 1) * NCH)
            nc.tensor.matmul(
                ps[:], bt_sb[:, msl], u_sb[:, nsl], start=False, stop=True
            )
            ob = ob_pool.tile([P, NCH], F16, tag="ob", name="ob")
            nc.scalar.activation(
                ob[:], ps[:], IDENT, bias=bias_sb[:, m : m + 1], scale=1.0 / SCALE
            )
            if m == MT - 1:
                e0, e1 = (nc.sync, nc.gpsimd) if n == 0 else (nc.sync, nc.scalar)
                HN = NCH // 2
                n0 = n * NCH
                e0.dma_start(out_d[msl, n0 : n0 + HN], ob[:, 0:HN])
                e1.dma_start(out_d[msl, n0 + HN : n0 + NCH], ob[:, HN:NCH])
            else:
                eng = nc.gpsimd if (2 * m + n) % 2 == 0 else nc.sync
                eng.dma_start(out_d[msl, nsl], ob[:])

        # k-interleaved warm start: each arriving k-tile immediately feeds
        # all leading (m, n) tiles, keeping the PE busy through the x load;
        # U (which needs all of x) runs after, before the first LoRA matmuls
        wps = {
            (m, n): mm_psum.tile([P, NCH], F32, tag="mm", name="mps")
            for m in range(WM)
            for n in range(NT)
        }
        for g in range(G8):
            for m in range(WM):
                w8t = w8_tiles[m]
                w_ap = w8t[:, g, :] if swil else w8t[:, g, :, :]
                for n in range(NT):
                    nsl = slice(n * NCH, (n + 1) * NCH)
                    nc.tensor.matmul(
                        wps[(m, n)][:],
                        w_ap,
                        xt8[g][:, nsl, :].rearrange("p t s -> p s t"),
                        start=g == 0,
                        stop=False,
                        perf_mode=DRMODE,
                    )
        for j in range(KT16):
            for m in range(WM):
                for n in range(NT):
                    nsl = slice(n * NCH, (n + 1) * NCH)
                    nc.tensor.matmul(
                        wps[(m, n)][:],
                        w16_tiles[m][:, j, :],
                        xt16[j][:, nsl],
                        start=(not G8) and j == 0,
                        stop=False,
                    )
        for m_ in range(WM, min(WM + 2, MT)):
            if G8:
                dma_w8(m_)
            if KT16:
                dma_w16(m_)
        emit_u(0)
        emit_u(1)
        for m in range(WM):
            for n in range(NT):
                emit_fin(m, n, wps[(m, n)])
        for m in range(WM, MT):
            for n in range(NT):
                emit_fin(m, n, emit_main(m, n))

    nc.compile()
    return nc


def _fp8(a):
    return np.clip(a, -240.0, 240.0).astype(FP8NP)


def make_in_maps(x, weight, bias, lora_A, lora_B, k8=K8, swil=SWIL):
    G8 = k8 // 256
    KT16 = (D_IN - k8) // P
    T = T_CORE
    xf = np.ascontiguousarray(x.reshape(-1, D_IN), dtype=np.float32)

    # ---- shared (per-core identical) weight-side arrays ----
    shared = {}
    w = np.asarray(weight, dtype=np.float32)
    a = np.asarray(lora_A, dtype=np.float32)
    if G8:
        wq8 = _fp8(SCALE * w[:, :k8]).reshape(MT, P, G8, 2, P)
        if swil:
            # [m, p, g, c_rev, s] pairs, contiguous for SW-interleaved ldweights
            shared["w8"] = np.ascontiguousarray(
                wq8[:, ::-1].transpose(0, 4, 2, 1, 3).reshape(MT, P, G8, 2 * P)
            )
        else:
            shared["w8"] = np.ascontiguousarray(wq8.transpose(0, 4, 2, 3, 1))
        aq8 = _fp8(SCALE * a[:, :k8]).T.reshape(G8, 2, P, R)
        shared["at8"] = np.ascontiguousarray(aq8.transpose(2, 0, 1, 3))
    if KT16:
        w16 = (SCALE * w[:, k8:]).astype(np.float16).reshape(MT, P, KT16, P)
        shared["w16"] = np.ascontiguousarray(w16.transpose(0, 3, 2, 1))
        a16 = (SCALE * a[:, k8:]).astype(np.float16).T.reshape(KT16, P, R)
        shared["at16"] = np.ascontiguousarray(a16.transpose(1, 0, 2))
    shared["bt"] = np.ascontiguousarray(
        np.asarray(lora_B, dtype=np.float32).T.astype(np.float16)
    )
    shared["bias_r"] = np.ascontiguousarray(
        np.asarray(bias, dtype=np.float32).reshape(MT, P).T
    )

    # ---- per-core token shards ----
    maps = []
    for c in range(N_CORES):
        xs = np.ascontiguousarray(xf[c * T : (c + 1) * T].T)  # [D_IN, T]
        m = dict(shared)
        if G8:
            m["x8"] = np.ascontiguousarray(
                _fp8(xs[:k8]).reshape(G8, 2, P, T).transpose(2, 0, 3, 1)
            )
        if KT16:
            m["x16"] = np.ascontiguousarray(
                xs[k8:].astype(np.float16).reshape(KT16, P, T).transpose(1, 0, 2)
            )
        maps.append(m)
    return maps


_nc_cache = {}


def kernel(x, weight, bias, lora_A, lora_B):
    key = (x.shape, weight.shape)
    if key not in _nc_cache:
        _nc_cache[key] = build_nc()
    nc = _nc_cache[key]
    in_maps = make_in_maps(x, weight, bias, lora_A, lora_B)
    res = bass_utils.run_bass_kernel_spmd(
        nc, in_maps, core_ids=list(range(N_CORES))
    )
    out = np.concatenate(
        [res.results[c]["out"].T.astype(np.float32) for c in range(N_CORES)], axis=0
    )
    return out.reshape(x.shape[:-1] + (weight.shape[0],))


if __name__ == "__main__":
    rng = np.random.default_rng(0)
    x = rng.standard_normal((B_FULL, S_FULL, D_IN), dtype=np.float32)
    w = (rng.standard_normal((D_OUT, D_IN), dtype=np.float32) * 0.02).astype(np.float32)
    b = (rng.standard_normal((D_OUT,), dtype=np.float32) * 0.02).astype(np.float32)
    la = (rng.standard_normal((R, D_IN), dtype=np.float32) * 0.02).astype(np.float32)
    lb = (rng.standard_normal((D_OUT, R), dtype=np.float32) * 0.02).astype(np.float32)
    out = kernel(x, w, b, la, lb)
    ref = x.reshape(-1, D_IN) @ (w + lb @ la).T + b
    err = np.abs(out.reshape(-1, D_OUT) - ref)
    denom = np.abs(ref).max()
    print("max abs err:", err.max(), "rel:", err.max() / denom)


# revision 40
# speedup vs baseline: 1.0140x; 1.0140x over previous
"""LoRA linear layer (out = x @ (W + B@A).T + bias) on 8 trn2 NeuronCores.

Strategy: data-parallel over tokens (B*S = 8192 -> 1024 tokens/core).
All layout work (transposes, dtype casts) happens on the host so the
device does nothing but matmuls:

  - x shard arrives pre-transposed: a fp8(e4m3) copy of the first K8
    contraction dims in DoubleRow pair layout, and a fp16 copy of the
    remaining dims. Both DMA straight into resident SBUF.
  - W arrives pre-transposed (k-major) and pre-cast: fp8 (DoubleRow
    [SW-interleave] pair layout, scaled by 64) for k < K8, fp16
    (scaled by 64) for k >= K8. Streamed per 128-row output block.
  - Per output block m and token chunk n: psum[o=128, t=512]
    accumulates G8 DoubleRow fp8 matmuls (256 k-dims each, 2x rate),
    KT16 fp16 matmuls (128 k-dims each), and one rank-16 LoRA matmul
    from B.T and U = (64*A) @ x.T.
  - Scalar engine evicts psum with the bias added and the 1/64 weight
    scale removed; plain contiguous DMA writes out.T [o, t] rows and
    the host transposes each shard back.

fp8 on only half the contraction dims keeps the max relative error
~1.8e-2 (measured on the reference data) while cutting PE time ~25%.
"""

import sys

sys.path.insert(0, "/opt/trn_rl_repo")

import numpy as np
import ml_dtypes

import concourse.bass as bass  # noqa: F401
import concourse.bacc as bacc
import concourse.tile as tile
from concourse import mybir, bass_utils
from contextlib import ExitStack

P = 128
N_CORES = 8

# Full problem shapes (hardcoded per contract).
B_FULL, S_FULL, D_IN, D_OUT, R = 4, 2048, 4096, 4096, 16
T_CORE = (B_FULL * S_FULL) // N_CORES  # 1024 tokens per core
MT = D_OUT // P  # 32 output row blocks
NCH = 512  # token chunk (one psum bank)
NT = T_CORE // NCH  # 2 chunks
K8 = 2304  # leading contraction dims done in fp8 DoubleRow
SCALE = 64.0  # fp8 weight scale (keeps 64*W in e4m3 normal range)
SWIL = True  # DoubleRowSwInterleave (contiguous ldweights) vs DoubleRow

FP8NP = ml_dtypes.float8_e4m3fn


def build_nc(T=T_CORE, k8=K8, swil=SWIL, fp16=None):
    """Per-core bass program; all cores run it on different token shards.

    fp16 kwarg is accepted for test.py compat: fp16=True/None keeps the
    default hybrid config, fp16-only can be forced with k8=0.
    """
    F32 = mybir.dt.float32
    F16 = mybir.dt.float16
    F8 = mybir.dt.float8e4
    G8 = k8 // 256
    KT16 = (D_IN - k8) // P
    DRMODE = (
        mybir.MatmulPerfMode.DoubleRowSwInterleave
        if swil
        else mybir.MatmulPerfMode.DoubleRow
    )
    IDENT = mybir.ActivationFunctionType.Identity
    HT = T // 2

    nc = bacc.Bacc("TRN2", target_bir_lowering=False, debug=False)
    if G8:
        # pair-interleaved moving layout: the two fp8 elements of each
        # DoubleRow pair sit adjacent in SBUF so the PE can stream both
        # per cycle
        x8_d = nc.dram_tensor("x8", [P, G8, T, 2], F8, kind="ExternalInput").ap()
        if swil:
            w8_d = nc.dram_tensor("w8", [MT, P, G8, 2 * P], F8, kind="ExternalInput").ap()
            at8_d = nc.dram_tensor("at8", [P, G8, 2 * P], F8, kind="ExternalInput").ap()
        else:
            w8_d = nc.dram_tensor("w8", [MT, P, G8, 2, P], F8, kind="ExternalInput").ap()
            at8_d = nc.dram_tensor("at8", [P, G8, 2, P], F8, kind="ExternalInput").ap()
    if KT16:
        x16_d = nc.dram_tensor("x16", [P, KT16, T], F16, kind="ExternalInput").ap()
        w16_d = nc.dram_tensor("w16", [MT, P, KT16, P], F16, kind="ExternalInput").ap()
        at16_d = nc.dram_tensor("at16", [P, KT16, R], F16, kind="ExternalInput").ap()
    bt_d = nc.dram_tensor("bt", [R, D_OUT], F16, kind="ExternalInput").ap()
    bias_d = nc.dram_tensor("bias_r", [P, MT], F32, kind="ExternalInput").ap()
    out_d = nc.dram_tensor("out", [D_OUT, T], F16, kind="ExternalOutput").ap()

    with tile.TileContext(nc) as tc, ExitStack() as ctx:
        const = ctx.enter_context(tc.tile_pool(name="const", bufs=1))
        if G8:
            # one tile per 256-dim k-group so matmuls only wait on their own
            # DMA slice, not the whole x transfer
            xt8 = [const.tile([P, T, 2], F8, name=f"x8g{g}") for g in range(G8)]
            at8s = const.tile([P, G8, 2 * P] if swil else [P, G8, 2, P], F8)
        if KT16:
            xt16 = [const.tile([P, T], F16, name=f"x16j{j}") for j in range(KT16)]
            at16s = const.tile([P, KT16, R], F16)
        bt_sb = const.tile([R, D_OUT], F16)
        bias_sb = const.tile([P, MT], F32)
        u_sb = const.tile([R, T], F16)

        up_psum = ctx.enter_context(tc.tile_pool(name="upps", bufs=2, space="PSUM"))
        mm_psum = ctx.enter_context(tc.tile_pool(name="mmps", bufs=6, space="PSUM"))
        w8_pool = ctx.enter_context(tc.tile_pool(name="w8p", bufs=8))
        w16_pool = ctx.enter_context(tc.tile_pool(name="w16p", bufs=8))
        ob_pool = ctx.enter_context(tc.tile_pool(name="obp", bufs=6))

        w8_tiles, w16_tiles = {}, {}

        def dma_w8(m):
            t8 = w8_pool.tile(
                [P, G8, 2 * P] if swil else [P, G8, 2, P], F8, tag="w8", name="w8t"
            )
            nc.scalar.dma_start(t8[:], w8_d[m])
            w8_tiles[m] = t8

        def dma_w16(m):
            t16 = w16_pool.tile([P, KT16, P], F16, tag="w16", name="w16t")
            nc.sync.dma_start(t16[:], w16_d[m])
            w16_tiles[m] = t16

        # ---- prologue DMAs: x split by k-range across the three queues so
        # every transfer keeps 2KB+ contiguous per-partition lines; the
        # first output block's matmuls chase the arriving k-tiles ----
        nc.sync.dma_start(bias_sb[:], bias_d[:])
        nc.sync.dma_start(bt_sb[:], bt_d[:])
        if KT16:
            nc.sync.dma_start(at16s[:], at16_d[:])
        WM = min(2, MT)
        if G8:
            nc.sync.dma_start(at8s[:], at8_d[:])
            for m_ in range(WM):
                dma_w8(m_)
        if KT16:
            for m_ in range(WM):
                dma_w16(m_)
        if G8:
            for g in range(G8 - 1):
                nc.scalar.dma_start(xt8[g][:], x8_d[:, g])
            nc.gpsimd.dma_start(xt8[G8 - 1][:], x8_d[:, G8 - 1])
        if KT16:
            JH = KT16 // 2
            for j in range(JH):
                nc.sync.dma_start(xt16[j][:], x16_d[:, j])
            for j in range(JH, KT16):
                nc.gpsimd.dma_start(xt16[j][:], x16_d[:, j])

        # ---- U = (64*A) @ x^T, [R, T], one token chunk ----
        def emit_u(n):
            nsl = slice(n * NCH, (n + 1) * NCH)
            # A.T is zero-padded to 128 stationary columns so the fp8 part
            # runs as DoubleRow (rank rows land in psum partitions 0..R-1)
            ups = up_psum.tile([P if G8 else R, NCH], F32, tag="up", name="ups")
            first = True
            for g in range(G8):
                at_ap = at8s[:, g, :] if swil else at8s[:, g, :, :]
                nc.tensor.matmul(
                    ups[:],
                    at_ap,
                    xt8[g][:, nsl, :].rearrange("p t s -> p s t"),
                    start=first,
                    stop=(not KT16) and g == G8 - 1,
                    perf_mode=DRMODE,
                )
                first = False
            for j in range(KT16):
                nc.tensor.matmul(
                    ups[0:R, :],
                    at16s[:, j, :],
                    xt16[j][:, nsl],
                    start=first,
                    stop=j == KT16 - 1,
                )
                first = False
            nc.vector.tensor_copy(u_sb[:, nsl], ups[0:R, :])

        # ---- one (m, n) output tile: 24 accumulating matmuls + eviction ----
        def emit_main(m, n):
            issued = w8_tiles if G8 else w16_tiles
            if n == 0:
                for mw in (m + 2, m + 3):
                    if mw < MT and mw not in issued:
                        if G8:
                            dma_w8(mw)
                        if KT16:
                            dma_w16(mw)
                        break
            nsl = slice(n * NCH, (n + 1) * NCH)
            ps = mm_psum.tile([P, NCH], F32, tag="mm", name="mps")
            for g in range(G8):
                w8t = w8_tiles[m]
                w_ap = w8t[:, g, :] if swil else w8t[:, g, :, :]
                nc.tensor.matmul(
                    ps[:],
                    w_ap,
                    xt8[g][:, nsl, :].rearrange("p t s -> p s t"),
                    start=g == 0,
                    stop=False,
                    perf_mode=DRMODE,
                )
            for j in range(KT16):
                nc.tensor.matmul(
                    ps[:],
                    w16_tiles[m][:, j, :],
                    xt16[j][:, nsl],
                    start=(not G8) and j == 0,
                    stop=False,
                )
            return ps

        def emit_fin(m, n, ps):
            msl = slice(m * P, (m + 1) * P)
            nsl = slice(n * NCH, (n + 1) * NCH)
            nc.tensor.matmul(
                ps[:], bt_sb[:, msl], u_sb[:, nsl], start=False, stop=True
            )
            ob = ob_pool.tile([P, NCH], F16, tag="ob", name="ob")
            nc.scalar.activation(
                ob[:], ps[:], IDENT, bias=bias_sb[:, m : m + 1], scale=1.0 / SCALE
            )
            if m == MT - 1:
                e0, e1 = (nc.sync, nc.gpsimd) if n == 0 else (nc.sync, nc.scalar)
                HN = NCH // 2
                n0 = n * NCH
                e0.dma_start(out_d[msl, n0 : n0 + HN], ob[:, 0:HN])
                e1.dma_start(out_d[msl, n0 + HN : n0 + NCH], ob[:, HN:NCH])
            else:
                eng = nc.gpsimd if (2 * m + n) % 2 == 0 else nc.sync
                eng.dma_start(out_d[msl, nsl], ob[:])

        # k-interleaved warm start: each arriving k-tile immediately feeds
        # all leading (m, n) tiles, keeping the PE busy through the x load;
        # U (which needs all of x) runs after, before the first LoRA matmuls
        wps = {
            (m, n): mm_psum.tile([P, NCH], F32, tag="mm", name="mps")
            for m in range(WM)
            for n in range(NT)
        }
        for g in range(G8):
            for m in range(WM):
                w8t = w8_tiles[m]
                w_ap = w8t[:, g, :] if swil else w8t[:, g, :, :]
                for n in range(NT):
                    nsl = slice(n * NCH, (n + 1) * NCH)
                    nc.tensor.matmul(
                        wps[(m, n)][:],
                        w_ap,
                        xt8[g][:, nsl, :].rearrange("p t s -> p s t"),
                        start=g == 0,
                        stop=False,
                        perf_mode=DRMODE,
                    )
        for j in range(KT16):
            for m in range(WM):
                for n in range(NT):
                    nsl = slice(n * NCH, (n + 1) * NCH)
                    nc.tensor.matmul(
                        wps[(m, n)][:],
                        w16_tiles[m][:, j, :],
                        xt16[j][:, nsl],
                        start=(not G8) and j == 0,
                        stop=False,
                    )
        for m_ in range(WM, min(WM + 2, MT)):
            if G8:
                dma_w8(m_)
            if KT16:
                dma_w16(m_)
        emit_u(0)
        emit_u(1)
        for m in range(WM):
            for n in range(NT):
                emit_fin(m, n, wps[(m, n)])
        for m in range(WM, MT):
            for n in range(NT):
                emit_fin(m, n, emit_main(m, n))

    nc.compile()
    return nc


def _fp8(a):
    return np.clip(a, -240.0, 240.0).astype(FP8NP)


def make_in_maps(x, weight, bias, lora_A, lora_B, k8=K8, swil=SWIL):
    G8 = k8 // 256
    KT16 = (D_IN - k8) // P
    T = T_CORE
    xf = np.ascontiguousarray(x.reshape(-1, D_IN), dtype=np.float32)

    # ---- shared (per-core identical) weight-side arrays ----
    shared = {}
    w = np.asarray(weight, dtype=np.float32)
    a = np.asarray(lora_A, dtype=np.float32)
    if G8:
        wq8 = _fp8(SCALE * w[:, :k8]).reshape(MT, P, G8, 2, P)
        a_pad = np.zeros((P, k8), dtype=np.float32)
        a_pad[:R] = SCALE * a[:, :k8]
        aq8 = _fp8(a_pad).reshape(1, P, G8, 2, P)
        if swil:
            # [m, p, g, c_rev, s] pairs, contiguous for SW-interleaved ldweights
            shared["w8"] = np.ascontiguousarray(
                wq8[:, ::-1].transpose(0, 4, 2, 1, 3).reshape(MT, P, G8, 2 * P)
            )
            shared["at8"] = np.ascontiguousarray(
                aq8[:, ::-1].transpose(0, 4, 2, 1, 3).reshape(P, G8, 2 * P)
            )
        else:
            shared["w8"] = np.ascontiguousarray(wq8.transpose(0, 4, 2, 3, 1))
            shared["at8"] = np.ascontiguousarray(
                aq8.transpose(0, 4, 2, 3, 1).reshape(P, G8, 2, P)
            )
    if KT16:
        w16 = (SCALE * w[:, k8:]).astype(np.float16).reshape(MT, P, KT16, P)
        shared["w16"] = np.ascontiguousarray(w16.transpose(0, 3, 2, 1))
        a16 = (SCALE * a[:, k8:]).astype(np.float16).T.reshape(KT16, P, R)
        shared["at16"] = np.ascontiguousarray(a16.transpose(1, 0, 2))
    shared["bt"] = np.ascontiguousarray(
        np.asarray(lora_B, dtype=np.float32).T.astype(np.float16)
    )
    shared["bias_r"] = np.ascontiguousarray(
        np.asarray(bias, dtype=np.float32).reshape(MT, P).T
    )

    # ---- per-core token shards ----
    maps = []
    for c in range(N_CORES):
        xs = np.ascontiguousarray(xf[c * T : (c + 1) * T].T)  # [D_IN, T]
        m = dict(shared)
        if G8:
            m["x8"] = np.ascontiguousarray(
                _fp8(xs[:k8]).reshape(G8, 2, P, T).transpose(2, 0, 3, 1)
            )
        if KT16:
            m["x16"] = np.ascontiguousarray(
                xs[k8:].astype(np.float16).reshape(KT16, P, T).transpose(1, 0, 2)
            )
        maps.append(m)
    return maps


_nc_cache = {}


def kernel(x, weight, bias, lora_A, lora_B):
    key = (x.shape, weight.shape)
    if key not in _nc_cache:
        _nc_cache[key] = build_nc()
    nc = _nc_cache[key]
    in_maps = make_in_maps(x, weight, bias, lora_A, lora_B)
    res = bass_utils.run_bass_kernel_spmd(
        nc, in_maps, core_ids=list(range(N_CORES))
    )
    out = np.concatenate(
        [res.results[c]["out"].T.astype(np.float32) for c in range(N_CORES)], axis=0
    )
    return out.reshape(x.shape[:-1] + (weight.shape[0],))


if __name__ == "__main__":
    rng = np.random.default_rng(0)
    x = rng.standard_normal((B_FULL, S_FULL, D_IN), dtype=np.float32)
    w = (rng.standard_normal((D_OUT, D_IN), dtype=np.float32) * 0.02).astype(np.float32)
    b = (rng.standard_normal((D_OUT,), dtype=np.float32) * 0.02).astype(np.float32)
    la = (rng.standard_normal((R, D_IN), dtype=np.float32) * 0.02).astype(np.float32)
    lb = (rng.standard_normal((D_OUT, R), dtype=np.float32) * 0.02).astype(np.float32)
    out = kernel(x, w, b, la, lb)
    ref = x.reshape(-1, D_IN) @ (w + lb @ la).T + b
    err = np.abs(out.reshape(-1, D_OUT) - ref)
    denom = np.abs(ref).max()
    print("max abs err:", err.max(), "rel:", err.max() / denom)
